# revision 4
# baseline (speedup 1.0000x reference)
"""CWAF block on 8 Trainium2 NeuronCores.

kernel(**inputs) takes the FULL unsharded inputs and returns (out, prob) like
the reference.  Work is sharded data-parallel over (batch, H-half): core
i = 2*b + half handles sample b, H rows [0,49) or [47,96) (one halo row for
the 3x3 reflect-pad conv).  Host-side W-flips (negative warp direction) and
H-flips (bottom half) make all 8 shard programs identical, so a single
pmap'd SPMD executable serves every core.

The compute runs on the NeuronCores through the PJRT (axon) backend as two
pmap'd XLA programs: a heavy pass (everything through the 3x3 conv, plus
per-shard channel sums) and a tiny tail (SE gate + ELU) that needs the
cross-shard global mean pool, combined on host between passes.  The
plane-sweep warp is expressed as 49 statically shifted views of the
edge-clamp-padded k row (bilinear sampling at a constant horizontal shift is
a 2-tap lerp of integer-shifted columns), so the shard lowers to dense
matmuls + elementwise ops with no gathers.
"""

import numpy as np
import functools

B, C, H, W = 4, 128, 96, 320
CR, D = 32, 24
TRAIN_W = 640.0
NORM_DISP = np.linspace(0.0, 0.3, D).astype(np.float32)
HH = H // 2          # 48 output rows per core
NR = HH + 1          # 49 rows loaded (1 halo)
J = 49               # integer taps

LAST_HW_NS = None


def _shift_consts(rel_scale):
    s = NORM_DISP.astype(np.float64) * 2.0 * rel_scale * (W - 1) / 2.0
    m = np.floor(s).astype(np.int64)
    m = np.minimum(m, J - 2)
    f = (s - m).astype(np.float32)
    return m, f


def _heavy_fn(rel_scale):
    import jax
    import jax.numpy as jnp

    m_np, f_np = _shift_consts(rel_scale)
    m_list = [int(v) for v in m_np]
    m1_list = [v + 1 for v in m_list]

    def heavy(t, s, qw, qb, kw, kb, cew, ceb, faw, fab, rw):
        # t, s: [C, NR, W] fp32 (already W/H-flipped on host as needed)
        q = jnp.einsum('oc,chw->ohw', qw, t) + qb[:, None, None]
        k = jnp.einsum('oc,chw->ohw', kw, s) + kb[:, None, None]
        q = q / jnp.maximum(jnp.sqrt(jnp.sum(q * q, 0, keepdims=True)), 1e-12)
        k = k / jnp.maximum(jnp.sqrt(jnp.sum(k * k, 0, keepdims=True)), 1e-12)
        # pad k on the right with its last column (border clamp), J-1 extra
        kp = jnp.concatenate([k, jnp.repeat(k[:, :, -1:], J - 1, axis=2)], axis=2)
        shifted = [jax.lax.dynamic_slice_in_dim(kp, j, W, 2) for j in range(J)]
        # 49-tap correlation volume: corr[h, w, j] = sum_c q[c,h,w] kp[c,h,w+j]
        corr = jnp.stack(
            [jnp.einsum('chw,chw->hw', q, sh) for sh in shifted], axis=-1)
        m = jnp.asarray(m_list)
        f = jnp.asarray(f_np)
        sim = (1.0 - f) * corr[..., m] + f * corr[..., jnp.asarray(m1_list)]
        prob = jax.nn.softmax(sim, axis=-1)        # [NR, W, D]
        # scatter prob into 49 integer-tap weights pw[h, w, j]
        pw = jnp.zeros(corr.shape, corr.dtype)
        pw = pw.at[..., m].add((1.0 - f) * prob)
        pw = pw.at[..., jnp.asarray(m1_list)].add(f * prob)
        cost = sum(sh * pw[None, :, :, j] for j, sh in enumerate(shifted))
        cost_exp = jnp.einsum('oc,chw->ohw', cew, cost) + ceb[:, None, None]
        alogit = (jnp.einsum('c,chw->hw', faw[:C], t)
                  + jnp.einsum('c,chw->hw', faw[C:], cost_exp) + fab)
        alpha = jax.nn.sigmoid(alogit)[None]
        fused = alpha * t + (1.0 - alpha) * cost_exp   # [C, NR, W]
        # reflect pad: top row -1 -> row 1 (host flips make every core "top");
        # bottom halo is row NR-1 (real neighbor data). W: reflect both sides.
        xp = jnp.concatenate([fused[:, 1:2, :], fused], axis=1)
        xp = jnp.concatenate([xp[:, :, 1:2], xp, xp[:, :, W - 2:W - 1]], axis=2)
        conv = jax.lax.conv_general_dilated(
            xp[None], rw, (1, 1), 'VALID',
            dimension_numbers=('NCHW', 'OIHW', 'NCHW'))[0]   # [C, HH, W]
        my_psum = jnp.sum(conv, axis=(1, 2))           # [C]
        return conv, prob[:HH].transpose(2, 0, 1), my_psum

    return heavy


def _tail_fn():
    import jax
    import jax.numpy as jnp

    def tail(conv, pooled, sw1, sw2):
        gate = jax.nn.sigmoid(sw2 @ jax.nn.relu(sw1 @ pooled))
        y = conv * gate[:, None, None]
        return jnp.where(y > 0, y, jnp.expm1(jnp.minimum(y, 0.0)))

    return tail


@functools.lru_cache(maxsize=8)
def _pmapped(rel_scale):
    import jax
    return (jax.pmap(_heavy_fn(rel_scale)), jax.pmap(_tail_fn()))


def kernel(t_feat, s_feat, directs, image_shape, q_w, q_b, k_w, k_b,
           ce_w, ce_b, fa_w, fa_b, redu_w, se_w1, se_w2):
    t_feat = np.asarray(t_feat, np.float32)
    s_feat = np.asarray(s_feat, np.float32)
    dirs = np.asarray(directs, np.float32).reshape(B)
    rel_scale = float(TRAIN_W / float(np.asarray(image_shape).reshape(-1)[3]))

    heavy, tail = _pmapped(rel_scale)

    faw = np.asarray(fa_w, np.float32).reshape(2 * C)
    fab = np.float32(np.asarray(fa_b).reshape(-1)[0])
    qw = np.asarray(q_w, np.float32); qb = np.asarray(q_b, np.float32)
    kw = np.asarray(k_w, np.float32); kb = np.asarray(k_b, np.float32)
    cew = np.asarray(ce_w, np.float32); ceb = np.asarray(ce_b, np.float32)
    sw1 = np.asarray(se_w1, np.float32); sw2 = np.asarray(se_w2, np.float32)
    rw_base = np.asarray(redu_w, np.float32)

    ts = np.empty((8, C, NR, W), np.float32)
    ss = np.empty((8, C, NR, W), np.float32)
    rws = np.empty((8,) + rw_base.shape, np.float32)
    meta = []
    for core in range(8):
        b, half = core // 2, core % 2
        wflip = bool(dirs[b] < 0)
        t = t_feat[b]; s = s_feat[b]
        if wflip:
            t = t[:, :, ::-1]; s = s[:, :, ::-1]
        if half == 0:
            t = t[:, :NR, :]; s = s[:, :NR, :]
        else:
            t = t[:, H - NR:, :][:, ::-1, :]; s = s[:, H - NR:, :][:, ::-1, :]
        rw = rw_base
        if wflip:
            rw = rw[:, :, :, ::-1]
        if half == 1:
            rw = rw[:, :, ::-1, :]
        ts[core] = t; ss[core] = s; rws[core] = rw
        meta.append((b, half, wflip))

    def bcast(a):
        a = np.asarray(a, np.float32)
        return np.broadcast_to(a, (8,) + a.shape)

    conv, prob_sh, psum = heavy(ts, ss, bcast(qw), bcast(qb), bcast(kw),
                                bcast(kb), bcast(cew), bcast(ceb), bcast(faw),
                                np.full((8,), fab, np.float32), rws)
    psum_np = np.asarray(psum)
    pooled = np.empty((8, C), np.float32)
    for core in range(8):
        pooled[core] = (psum_np[core] + psum_np[core ^ 1]) / np.float32(H * W)

    out_sh = np.asarray(tail(conv, pooled, bcast(sw1), bcast(sw2)))
    prob_sh = np.asarray(prob_sh)

    out = np.empty((B, C, H, W), np.float32)
    prob = np.empty((B, D, H, W), np.float32)
    for core in range(8):
        b, half, wflip = meta[core]
        o = out_sh[core]; p = prob_sh[core]
        if half == 1:
            o = o[:, ::-1, :]; p = p[:, ::-1, :]
        if wflip:
            o = o[:, :, ::-1]; p = p[:, :, ::-1]
        rows = slice(0, HH) if half == 0 else slice(HH, H)
        out[b, :, rows] = o
        prob[b, :, rows] = p
    return out, prob
